# revision 1
# baseline (speedup 1.0000x reference)
"""AttentiveDensenet Trainium2 Bass kernel.

Data-parallel over batch B=8 across 8 NeuronCores (1 image per core).
Per layer l (of 4):
  - K/Q/V 1x1 convs as bf16 matmuls with x-tiles as the stationary operand,
    producing position-major [pos, (head, dim)] activations directly
    (avoids any transpose for the attention stage). Bias via a K=1
    ones-row matmul accumulated into PSUM.
  - Attention is per-token over the growing key/val list: score products on
    DVE (bf16, 2x mode), d-reduction on DVE, softmax + exact top-k
    (2nd-smallest via min-of-pairwise-max) on DVE/ACT, weighted sum on
    GPSIMD (products) + DVE (accumulate).
  - o is cast to bf16, bounced through DRAM, and transposed to channel-major
    padded layout with the DMA xbar transpose engine.
  - conv3x3 #1 as 9 shifted 1x1 convs accumulated in PSUM (bf16).
  - BatchNorm stats: per-core per-channel sum/sumsq, AllGathered across the
    8 cores (2KB), summed locally. Training-mode BN; the conv bias ob1
    cancels exactly in BN and is dropped.
  - h1 = relu(A*y1 + B) fused on the scalar engine, written bf16 into the
    padded conv2 input.
  - conv3x3 #2 (bf16) + residual x += gamma*(h2 + ob2) fused via
    scalar_tensor_tensor from PSUM.
"""
import numpy as np
import ml_dtypes

import concourse.bacc as bacc
import concourse.mybir as mybir
import concourse.tile as tile
from concourse import bass_utils

L, C, B, H, W = 4, 256, 8, 32, 32
NH, KD = 8, 64
KH = NH * KD          # 512
HW = H * W            # 1024
P = 128
NC = 8                # cores
TOPK = 4
EPS = 1e-7
BN_EPS = 1e-5
PW = W + 2            # 34
PHW = PW * (H + 2)    # 1156

f32 = mybir.dt.float32
bf16 = mybir.dt.bfloat16
AX = mybir.AxisListType
OP = mybir.AluOpType
ACTF = mybir.ActivationFunctionType

_compiled = {}
DBGL = 0


def _build(ncores=NC, dbg=False, no_cc=False, no_gps=False, no_xpose=False, dense_rhs=False, layers=L, stages=99):
    nc = bacc.Bacc(None, target_bir_lowering=False, debug=False, num_devices=ncores)

    # ---- DRAM I/O (per-core shapes) ----
    xin = nc.dram_tensor("xin", [C, HW], f32, kind="ExternalInput").ap()
    wq = nc.dram_tensor("wq", [L, 2, P, KH], bf16, kind="ExternalInput").ap()
    wk = nc.dram_tensor("wk", [L, 2, P, KH], bf16, kind="ExternalInput").ap()
    wv = nc.dram_tensor("wv", [L, 2, P, KH], bf16, kind="ExternalInput").ap()
    bq = nc.dram_tensor("bq", [L, 1, KH], bf16, kind="ExternalInput").ap()
    bk = nc.dram_tensor("bk", [L, 1, KH], bf16, kind="ExternalInput").ap()
    bv = nc.dram_tensor("bv", [L, 1, KH], bf16, kind="ExternalInput").ap()
    w1 = nc.dram_tensor("w1", [L, 9, 4, 2, P, P], bf16, kind="ExternalInput").ap()
    w2 = nc.dram_tensor("w2", [L, 9, 2, 2, P, P], bf16, kind="ExternalInput").ap()
    bngd = nc.dram_tensor("bngd", [L, 2, P, 1], f32, kind="ExternalInput").ap()
    bnbd = nc.dram_tensor("bnbd", [L, 2, P, 1], f32, kind="ExternalInput").ap()
    gob2d = nc.dram_tensor("gob2d", [L, 2, P, 1], f32, kind="ExternalInput").ap()
    gamd = nc.dram_tensor("gamd", [L, P, 1], f32, kind="ExternalInput").ap()
    out = nc.dram_tensor("out", [C, HW], f32, kind="ExternalOutput").ap()
    dbgt = {}
    if dbg:
        for nm, shp in [("d_q", [P, 8 * KH]), ("d_k", [P, 8 * KH]), ("d_v", [P, 8 * KH]),
                        ("d_S", [P, 320]), ("d_attn", [P, 320]), ("d_o", [P, 8 * KH]),
                        ("d_opad0", [P, PHW]), ("d_y1_0", [P, HW]), ("d_gsum", [P, 4]),
                        ("d_A0", [P, 1]), ("d_B0", [P, 1]), ("d_h1p0", [P, PHW]),
                        ("d_x0", [P, HW])]:
            dbgt[nm] = nc.dram_tensor(nm, shp, f32, kind="ExternalOutput").ap()

    with tile.TileContext(nc) as tc:
        with tc.tile_pool(name="main", bufs=1) as mp, \
             tc.tile_pool(name="prodp", bufs=2) as prodp, \
             tc.tile_pool(name="tmpp", bufs=2) as tmpp, \
             tc.tile_pool(name="wkvp", bufs=4) as wkvp, \
             tc.tile_pool(name="wcp", bufs=12) as wcp, \
             tc.tile_pool(name="biasp", bufs=3) as biasp, \
             tc.tile_pool(name="kqvps", bufs=4, space="PSUM") as kqvps, \
             tc.tile_pool(name="convps", bufs=4, space="PSUM") as convps, \
             tc.tile_pool(name="dramp", bufs=2, space="DRAM") as dramp:

            # persistent tiles
            x = [mp.tile([P, HW], f32, name=f"x{i}") for i in range(2)]
            xb = [mp.tile([P, HW], bf16, name=f"xb{i}") for i in range(2)]
            qbt = mp.tile([P, 8 * KH], bf16, name="qbt")
            kbt = [mp.tile([P, 8 * KH], bf16, name=f"kbt{i}") for i in range(L)]
            vbt = [mp.tile([P, 8 * KH], bf16, name=f"vbt{i}") for i in range(L)]
            S = mp.tile([P, 64 * 5], f32, name="S")
            attn = mp.tile([P, 64 * 5], f32, name="attn")
            attnb = mp.tile([P, 64 * 5], bf16, name="attnb")
            mx = mp.tile([P, 64], f32, name="mx")
            zs = mp.tile([P, 64], f32, name="zs")
            dmin = mp.tile([P, 64], f32, name="dmin")
            mxp = mp.tile([P, 64], f32, name="mxp")
            o = mp.tile([P, 8 * KH], f32, name="o")
            obf = mp.tile([P, 8 * KH], bf16, name="obf")
            opad = [mp.tile([P, PHW + 2], bf16, name=f"opad{i}") for i in range(4)]
            y1 = [mp.tile([P, HW], f32, name=f"y1_{i}") for i in range(2)]
            h1p = [mp.tile([P, PHW + 2], bf16, name=f"h1p{i}") for i in range(2)]
            st = mp.tile([P, 4], f32, name="st")
            gst = mp.tile([P, 32], f32, name="gst")
            gsum = mp.tile([P, 4], f32, name="gsum")
            ones1 = mp.tile([1, P], bf16, name="ones1")
            # per-layer consts (reloaded each layer)
            bngt = [mp.tile([P, 1], f32, name=f"bngt{i}") for i in range(2)]
            bnbt = [mp.tile([P, 1], f32, name=f"bnbt{i}") for i in range(2)]
            gob2t = [mp.tile([P, 1], f32, name=f"gob2t{i}") for i in range(2)]
            gamt = mp.tile([P, 1], f32, name="gamt")
            # BN scratch
            t1 = [mp.tile([P, 1], f32, name=f"t1_{i}") for i in range(2)]
            Ac = [mp.tile([P, 1], f32, name=f"Ac{i}") for i in range(2)]
            Bc = [mp.tile([P, 1], f32, name=f"Bc{i}") for i in range(2)]
            sq = mp.tile([P, 1], f32, name="sq")
            vart = mp.tile([P, 1], f32, name="vart")
            stdt = mp.tile([P, 1], f32, name="stdt")

            # init
            for i in range(2):
                nc.sync.dma_start(x[i][:], xin[i * P:(i + 1) * P, :])
                nc.scalar.copy(xb[i][:], x[i][:])
            for i in range(4):
                nc.vector.memset(opad[i][:], 0)
            for i in range(2):
                nc.vector.memset(h1p[i][:], 0)
            nc.vector.memset(ones1[:], 1.0)
            nc.vector.memset(S[:], 0)
            nc.vector.memset(attn[:], 0)

            S3 = S[:].rearrange("p (g t) -> p g t", t=5)
            at3 = attn[:].rearrange("p (g t) -> p g t", t=5)
            ab3 = attnb[:].rearrange("p (g t) -> p g t", t=5)

            for l in range(layers):
                R = l + 1      # number of real keys
                T = R + 1      # +1 zero key

                # ---- per-layer consts ----
                for i in range(2):
                    nc.sync.dma_start(bngt[i][:], bngd[l, i])
                    nc.sync.dma_start(bnbt[i][:], bnbd[l, i])
                    nc.sync.dma_start(gob2t[i][:], gob2d[l, i])
                nc.sync.dma_start(gamt[:], gamd[l])

                # ---- K/Q/V 1x1 convs, position-major ----
                for name, wdr, bdr, dest in (
                    ("k", wk, bk, kbt[l][:]),
                    ("v", wv, bv, vbt[l][:]),
                    ("q", wq, bq, qbt[:]),
                ):
                    bt = biasp.tile([1, KH], bf16, name=f"bias_{name}_{l}", tag="bias")
                    nc.sync.dma_start(bt[:], bdr[l])
                    wts = []
                    for ct in range(2):
                        wt = wkvp.tile([P, KH], bf16, name=f"w_{name}_{l}_{ct}", tag="wkv")
                        nc.sync.dma_start(wt[:], wdr[l, ct])
                        wts.append(wt)
                    for pb in range(8):
                        ps = kqvps.tile([P, KH], f32, name="kqv_ps")
                        nc.tensor.matmul(ps[:], ones1[:], bt[:], start=True, stop=False)
                        nc.tensor.matmul(ps[:], xb[0][:, pb * P:(pb + 1) * P], wts[0][:],
                                         start=False, stop=False)
                        nc.tensor.matmul(ps[:], xb[1][:, pb * P:(pb + 1) * P], wts[1][:],
                                         start=False, stop=True)
                        nc.scalar.copy(dest[:, pb * KH:(pb + 1) * KH], ps[:])

                # ---- scores ----
                if stages < 2: continue
                for t in range(R):
                    pr = prodp.tile([P, 8 * KH], bf16, name="prodb")
                    nc.vector.tensor_mul(pr[:], qbt[:], kbt[t][:])
                    nc.vector.tensor_reduce(
                        out=S3[:, :, t], in_=pr[:].rearrange("p (g d) -> p g d", d=KD),
                        axis=AX.X, op=OP.add)
                nc.vector.memset(S3[:, :, R:R + 1], 0)  # zero key

                # ---- softmax over T slots ----
                if stages < 3: continue
                nc.vector.tensor_reduce(out=mx[:], in_=S3[:, :, 0:T], axis=AX.X, op=OP.max)
                nc.vector.tensor_tensor(
                    at3[:, :, 0:T], S3[:, :, 0:T],
                    mx[:].unsqueeze(2).broadcast_to([P, 64, T]), OP.subtract)
                nc.scalar.activation(at3[:, :, 0:T], at3[:, :, 0:T], ACTF.Exp)
                nc.vector.tensor_reduce(out=zs[:], in_=at3[:, :, 0:T], axis=AX.X, op=OP.add)
                nc.vector.reciprocal(zs[:], zs[:])
                nc.vector.tensor_tensor(
                    at3[:, :, 0:T], at3[:, :, 0:T],
                    zs[:].unsqueeze(2).broadcast_to([P, 64, T]), OP.mult)

                # ---- sparse top-k (only T=5) ----
                if T > TOPK:
                    first = True
                    for i in range(T):
                        for j in range(i + 1, T):
                            dst = dmin if first else mxp
                            nc.vector.tensor_tensor(
                                dst[:], at3[:, :, i], at3[:, :, j],
                                OP.max)
                            if not first:
                                nc.vector.tensor_tensor(dmin[:], dmin[:], mxp[:], OP.min)
                            first = False
                    nc.vector.tensor_scalar_add(dmin[:], dmin[:], EPS)
                    nc.vector.tensor_tensor(
                        at3[:, :, 0:T], at3[:, :, 0:T],
                        dmin[:].unsqueeze(2).broadcast_to([P, 64, T]), OP.subtract)
                    nc.vector.tensor_scalar_max(at3[:, :, 0:T], at3[:, :, 0:T], 0.0)
                    nc.vector.tensor_reduce(out=zs[:], in_=at3[:, :, 0:T], axis=AX.X,
                                            op=OP.add)
                    nc.vector.tensor_scalar_add(zs[:], zs[:], EPS)
                    nc.vector.reciprocal(zs[:], zs[:])
                    nc.vector.tensor_tensor(
                        at3[:, :, 0:T], at3[:, :, 0:T],
                        zs[:].unsqueeze(2).broadcast_to([P, 64, T]), OP.mult)

                nc.vector.tensor_copy(attnb[:], attn[:])

                # ---- weighted sum: o = sum_t attn_t * v_t ----
                if stages < 4: continue
                o3 = o[:].rearrange("p (g d) -> p g d", d=KD)
                for t in range(R):
                    v3 = vbt[t][:].rearrange("p (g d) -> p g d", d=KD)
                    ab = ab3[:, :, t].unsqueeze(2).broadcast_to([P, 64, KD])
                    eng = nc.vector
                    if t == 0:
                        eng.tensor_tensor(o3, v3, ab, OP.mult)
                    else:
                        tm = tmpp.tile([P, 8 * KH], bf16, name="wtmp")
                        tm3 = tm[:].rearrange("p (g d) -> p g d", d=KD)
                        eng.tensor_tensor(tm3, v3, ab, OP.mult)
                        nc.vector.tensor_add(o[:], o[:], tm[:])

                # ---- o -> bf16 -> DRAM -> xbar transpose -> opad ----
                if stages < 5: continue
                nc.scalar.copy(obf[:], o[:])
                odr = dramp.tile([8 * P, KH], bf16, name="odr")
                nc.sync.dma_start(
                    odr[:].rearrange("(b r) h -> r b h", r=P),
                    obf[:].rearrange("p (b h) -> p b h", h=KH))
                for ht in range(4):
                    obt = tmpp.tile([P, HW], bf16, name="obt", tag="obt")
                    if no_xpose:
                        nc.sync.dma_start(obt[:].rearrange('p (a b) -> p a b', b=KH), odr[0:P * 2].rearrange('(p a) h -> p a h', p=P))
                    else:
                        nc.sync.dma_start_transpose(obt[:], odr[:, ht * P:(ht + 1) * P])
                    opv = opad[ht][:, 0:PHW].rearrange("c (i j) -> c i j", j=PW)
                    nc.sync.dma_start(
                        opv[:, 1:H + 1, 1:W + 1],
                        obt[:].rearrange("c (i j) -> c i j", j=W))

                # ---- conv3x3 #1 (bf16): y1 = W1 * opad ----
                if stages < 6: continue
                CHUNKS = [(0, 15), (15, 15), (30, 2)]
                for co in range(2):
                    for (i0, nr) in CHUNKS:
                        ps = convps.tile([P, 512], f32, name="c1ps", tag="cps")
                        nw = PW * nr
                        for tap in range(9):
                            ty, tx = tap // 3, tap % 3
                            for ci in range(4):
                                wt = wcp.tile([P, P], bf16, name="w1t")
                                nc.sync.dma_start(wt[:], w1[l, tap, ci, co])
                                base = PW * (i0 + ty) + tx
                                nc.tensor.matmul(
                                    ps[:, 0:nw], wt[:], opad[ci][:, base:base + nw],
                                    start=(tap == 0 and ci == 0),
                                    stop=(tap == 8 and ci == 3))
                        nc.scalar.copy(
                            y1[co][:, W * i0:W * (i0 + nr)].rearrange(
                                "c (i j) -> c i j", j=W),
                            ps[:, 0:nw].rearrange("c (i j) -> c i j", j=PW)[:, :, 0:W])

                # ---- BN stats + AllGather ----
                if stages < 7: continue
                for co in range(2):
                    nc.vector.tensor_reduce(out=st[:, 2 * co:2 * co + 1], in_=y1[co][:],
                                            axis=AX.X, op=OP.add)
                    nc.scalar.square(o[:, 0:HW], y1[co][:])
                    nc.vector.tensor_reduce(out=st[:, 2 * co + 1:2 * co + 2],
                                            in_=o[:, 0:HW], axis=AX.X, op=OP.add)
                if no_cc:
                    nc.vector.tensor_scalar_mul(gsum[:], st[:], float(ncores))
                else:
                    cci = dramp.tile([1, 512], f32, name="cci")
                    cco = dramp.tile([ncores, 512], f32, name="cco", addr_space="Shared")
                    nc.sync.dma_start(cci[0].rearrange("(p j) -> p j", j=4), st[:])
                    nc.gpsimd.collective_compute(
                        "AllGather", OP.bypass,
                        replica_groups=[list(range(ncores))],
                        ins=[cci.opt()], outs=[cco.opt()])
                    nc.sync.dma_start(
                        gst[:, 0:4 * ncores].rearrange("p (j s) -> p j s", s=ncores),
                        cco[:].rearrange("s (p j) -> p j s", j=4))
                    nc.vector.tensor_reduce(
                        out=gsum[:], in_=gst[:, 0:4 * ncores].rearrange("p (j s) -> p j s", s=ncores),
                        axis=AX.X, op=OP.add)

                # ---- BN coefficients: A = g/sqrt(var+eps), B = b - mean*A ----
                if stages < 8: continue
                NTOT = float(ncores * HW)
                for co in range(2):
                    nc.vector.tensor_scalar_mul(t1[co][:], gsum[:, 2 * co:2 * co + 1],
                                                1.0 / NTOT)
                    nc.vector.tensor_scalar_mul(vart[:], gsum[:, 2 * co + 1:2 * co + 2],
                                                1.0 / NTOT)
                    nc.vector.tensor_mul(sq[:], t1[co][:], t1[co][:])
                    nc.vector.tensor_sub(vart[:], vart[:], sq[:])
                    nc.vector.tensor_scalar_add(vart[:], vart[:], BN_EPS)
                    nc.scalar.activation(stdt[:], vart[:], ACTF.Sqrt)
                    nc.vector.reciprocal(stdt[:], stdt[:])
                    nc.vector.tensor_mul(Ac[co][:], bngt[co][:], stdt[:])
                    nc.vector.tensor_mul(sq[:], t1[co][:], Ac[co][:])
                    nc.vector.tensor_sub(Bc[co][:], bnbt[co][:], sq[:])
                    # h1 = relu(A*y1 + B), strided bf16 into padded conv2 input
                    h1v = h1p[co][:, 0:PHW].rearrange("c (i j) -> c i j", j=PW)
                    nc.scalar.activation(
                        h1v[:, 1:H + 1, 1:W + 1],
                        y1[co][:].rearrange("c (i j) -> c i j", j=W),
                        ACTF.Relu, bias=Bc[co][:], scale=Ac[co][:])

                # ---- conv3x3 #2 (bf16) + residual update ----
                if stages < 9: continue
                for co in range(2):
                    nc.scalar.add(x[co][:], x[co][:], gob2t[co][:])
                    for (i0, nr) in CHUNKS:
                        ps = convps.tile([P, 512], f32, name="c2ps", tag="cps")
                        nw = PW * nr
                        for tap in range(9):
                            ty, tx = tap // 3, tap % 3
                            for ci in range(2):
                                wt = wcp.tile([P, P], bf16, name="w1t")
                                nc.sync.dma_start(wt[:], w2[l, tap, ci, co])
                                base = PW * (i0 + ty) + tx
                                nc.tensor.matmul(
                                    ps[:, 0:nw], wt[:], h1p[ci][:, base:base + nw],
                                    start=(tap == 0 and ci == 0),
                                    stop=(tap == 8 and ci == 1))
                        xslice = x[co][:, W * i0:W * (i0 + nr)]
                        nc.vector.scalar_tensor_tensor(
                            out=xslice.rearrange("c (i j) -> c i j", j=W),
                            in0=ps[:, 0:nw].rearrange("c (i j) -> c i j", j=PW)[:, :, 0:W],
                            scalar=gamt[:],
                            in1=xslice.rearrange("c (i j) -> c i j", j=W),
                            op0=OP.mult, op1=OP.add)
                    if l < layers - 1:
                        nc.scalar.copy(xb[co][:], x[co][:])
                    else:
                        nc.sync.dma_start(out[co * P:(co + 1) * P, :], x[co][:])
                if dbg and l == DBGL:
                    fcvt = mp.tile([P, 8 * KH], f32, name="fcvt")
                    for nm, src_t in [("d_q", qbt), ("d_k", kbt[l]), ("d_v", vbt[l]),
                                      ("d_o", o)]:
                        nc.vector.tensor_copy(fcvt[:], src_t[:])
                        nc.sync.dma_start(dbgt[nm], fcvt[:])
                    nc.vector.tensor_copy(fcvt[:, 0:320], S[:])
                    nc.sync.dma_start(dbgt["d_S"], fcvt[:, 0:320])
                    nc.vector.tensor_copy(fcvt[:, 0:320], attn[:])
                    nc.sync.dma_start(dbgt["d_attn"], fcvt[:, 0:320])
                    nc.vector.tensor_copy(fcvt[:, 0:PHW], opad[0][:, 0:PHW])
                    nc.sync.dma_start(dbgt["d_opad0"], fcvt[:, 0:PHW])
                    nc.sync.dma_start(dbgt["d_y1_0"], y1[0][:])
                    nc.sync.dma_start(dbgt["d_gsum"], gsum[:])
                    nc.sync.dma_start(dbgt["d_A0"], Ac[0][:])
                    nc.sync.dma_start(dbgt["d_B0"], Bc[0][:])
                    nc.vector.tensor_copy(fcvt[:, 0:PHW], h1p[0][:, 0:PHW])
                    nc.sync.dma_start(dbgt["d_h1p0"], fcvt[:, 0:PHW])
                    nc.sync.dma_start(dbgt["d_x0"], x[0][:])

    nc.compile()
    return nc


def _host_prep(inputs):
    bf = ml_dtypes.bfloat16
    kw, kb, qw, qb = inputs["kw"], inputs["kb"], inputs["qw"], inputs["qb"]
    vw, vb = inputs["vw"], inputs["vb"]
    ow1, ow2 = inputs["ow1"], inputs["ow2"]
    gammas, ob2 = inputs["gammas"], inputs["ob2"]

    def packw(wm):  # [L, KH, C] -> [L, 2, 128, KH]
        return np.ascontiguousarray(
            wm.transpose(0, 2, 1).reshape(L, 2, P, KH)).astype(bf)

    d = {}
    d["wq"] = packw(qw / 8.0)
    d["wk"] = packw(kw)
    d["wv"] = packw(vw)
    d["bq"] = np.ascontiguousarray((qb / 8.0).reshape(L, 1, KH)).astype(bf)
    d["bk"] = np.ascontiguousarray(kb.reshape(L, 1, KH)).astype(bf)
    d["bv"] = np.ascontiguousarray(vb.reshape(L, 1, KH)).astype(bf)
    # ow1 [L, 256, 512, 3, 3] -> [L, tap, ci(4), co(2), a(cin128), b(cout128)]
    a1 = ow1.reshape(L, 2, P, 4, P, 3, 3).transpose(0, 5, 6, 3, 1, 4, 2)
    d["w1"] = np.ascontiguousarray(a1.reshape(L, 9, 4, 2, P, P)).astype(bf)
    a2 = ow2.reshape(L, 2, P, 2, P, 3, 3).transpose(0, 5, 6, 3, 1, 4, 2)
    d["w2"] = np.ascontiguousarray(a2.reshape(L, 9, 2, 2, P, P)).astype(bf)
    d["bngd"] = np.ascontiguousarray(
        inputs["bn_g"].reshape(L, 2, P, 1)).astype(np.float32)
    d["bnbd"] = np.ascontiguousarray(
        inputs["bn_b"].reshape(L, 2, P, 1)).astype(np.float32)
    gob2 = gammas[:, None] * ob2
    d["gob2d"] = np.ascontiguousarray(gob2.reshape(L, 2, P, 1)).astype(np.float32)
    d["gamd"] = np.ascontiguousarray(
        np.broadcast_to(gammas[:, None, None], (L, P, 1))).astype(np.float32)
    return d


def kernel(**inputs):
    if "nc" not in _compiled:
        _compiled["nc"] = _build()
    nc = _compiled["nc"]
    shared = _host_prep(inputs)
    x = np.ascontiguousarray(inputs["x"].reshape(B, C, HW)).astype(np.float32)
    in_maps = []
    for c in range(NC):
        m = dict(shared)
        m["xin"] = x[c]
        in_maps.append(m)
    res = bass_utils.run_bass_kernel_spmd(nc, in_maps, core_ids=list(range(NC)))
    outs = np.stack([res.results[c]["out"] for c in range(NC)])
    return outs.reshape(B, C, H, W).astype(np.float32)



# revision 12
# speedup vs baseline: 2.3025x; 2.3025x over previous
"""AttentiveDensenet Trainium2 Bass kernel (v2).

Data-parallel over batch B=8 across 8 NeuronCores (1 image per core).

Key structure (vs v1): all conv/KQV weights are DMA'd in ONE batched
transfer per (layer, tensor) from host-packed layouts (the v1 per-tile
weight DMAs cost ~850us of sync-sequencer DIRECT2D issue time and
starved the PE into mid-pstate). The o pos-major -> channel-major
transpose is done on the PE (is_transpose matmuls against an identity)
instead of a DRAM xbar bounce. Attention arithmetic runs in bf16
(DVE 2x mode; validated 1.34e-2 rel err vs 2e-2 gate), split between
DVE and GpSimd, and is processed in two position-halves so conv1
chunk 0 and the o-transpose of half 0 overlap with attention of
half 1. BN AllGathers (one per channel-half) overlap conv1 of the
other half and conv2's ci=0 taps via a channel-half-split pipeline.

Per layer l (of 4):
  - K/Q/V 1x1 convs as bf16 matmuls, x-tiles stationary, position-major
    [pos, (head, dim)] output; bias via ones-row matmul into PSUM.
  - scores/softmax/top-k/weighted-sum on DVE (+GpSimd offload), bf16.
  - o transposed on PE into zero-padded conv input tiles.
  - conv3x3 #1 as 9 shifted 1x1 matmuls accumulated in PSUM.
  - BN stats (sum/sumsq) AllGathered across 8 cores (2KB); ob1 cancels
    in training-mode BN and is dropped. h1 = relu(A*y1+B) fused on Act.
  - conv3x3 #2 + residual x += gamma*(h2 + ob2) via scalar_tensor_tensor.
"""
import numpy as np
import ml_dtypes

import concourse.bacc as bacc
import concourse.mybir as mybir
import concourse.tile as tile
from concourse import bass_utils

L, C, B, H, W = 4, 256, 8, 32, 32
NH, KD = 8, 64
KH = NH * KD          # 512
HW = H * W            # 1024
P = 128
NC = 8                # cores
TOPK = 4
EPS = 1e-7
BN_EPS = 1e-5
PW = W + 2            # 34
PHW = PW * (H + 2)    # 1156
CHUNKS = [(0, 15), (15, 15), (30, 2)]

f32 = mybir.dt.float32
bf16 = mybir.dt.bfloat16
AX = mybir.AxisListType
OP = mybir.AluOpType
ACTF = mybir.ActivationFunctionType

_compiled = {}


def _build(ncores=NC, layers=L):
    nc = bacc.Bacc(None, target_bir_lowering=False, debug=False, num_devices=ncores)

    xin = nc.dram_tensor("xin", [C, HW], f32, kind="ExternalInput").ap()
    wkqvd = nc.dram_tensor("wkqvd", [L, P, 3 * 2 * KH], bf16, kind="ExternalInput").ap()
    bkd = nc.dram_tensor("bkd", [L, 1, 3 * KH], bf16, kind="ExternalInput").ap()
    w1d = nc.dram_tensor("w1d", [L, P, 72 * P], bf16, kind="ExternalInput").ap()
    w2d = nc.dram_tensor("w2d", [L, P, 36 * P], bf16, kind="ExternalInput").ap()
    cstd = nc.dram_tensor("cstd", [L, P, 8], f32, kind="ExternalInput").ap()
    identd = nc.dram_tensor("identd", [P, P], bf16, kind="ExternalInput").ap()
    out = nc.dram_tensor("out", [C, HW], f32, kind="ExternalOutput").ap()

    with tile.TileContext(nc) as tc, \
         nc.allow_low_precision(reason="bf16 attention validated vs reference"):
        with tc.tile_pool(name="main", bufs=1) as mp, \
             tc.tile_pool(name="prodp", bufs=4) as prodp, \
             tc.tile_pool(name="kqvps", bufs=2, space="PSUM") as kqvps, \
             tc.tile_pool(name="tpps", bufs=2, space="PSUM") as tpps, \
             tc.tile_pool(name="convps", bufs=3, space="PSUM") as convps, \
             tc.tile_pool(name="dramp", bufs=4, space="DRAM") as dramp:

            # ---- persistent tiles ----
            x = [mp.tile([P, HW], f32, name=f"x{i}") for i in range(2)]
            xb = [mp.tile([P, HW], bf16, name=f"xb{i}") for i in range(2)]
            qbt = mp.tile([P, 8 * KH], bf16, name="qbt")
            kbt = [mp.tile([P, 8 * KH], bf16, name=f"kbt{i}") for i in range(L)]
            vbt = [mp.tile([P, 8 * KH], bf16, name=f"vbt{i}") for i in range(L)]
            obf = mp.tile([P, 8 * KH], bf16, name="obf")
            S = mp.tile([P, 5 * 64], bf16, name="S")
            attnb = mp.tile([P, 5 * 64], bf16, name="attnb")
            mx = mp.tile([P, 64], bf16, name="mx")
            zs = mp.tile([P, 64], f32, name="zs")
            dmin = mp.tile([P, 64], bf16, name="dmin")
            mxp = mp.tile([P, 64], bf16, name="mxp")
            opad = [mp.tile([P, PHW + 2], bf16, name=f"opad{i}") for i in range(4)]
            y1 = [mp.tile([P, HW], f32, name=f"y1_{i}") for i in range(2)]
            sqs = [mp.tile([P, HW], f32, name=f"sqs{i}") for i in range(2)]
            h1p = [mp.tile([P, PHW + 2], bf16, name=f"h1p{i}") for i in range(2)]
            st = mp.tile([P, 4], f32, name="st")
            gst = mp.tile([P, 32], f32, name="gst")
            gsum = mp.tile([P, 4], f32, name="gsum")
            ones1 = mp.tile([1, P], bf16, name="ones1")
            ident = mp.tile([P, P], bf16, name="ident")
            # weight buffers (single-buffered; each reloads for layer l+1
            # right after its last layer-l consumer, hidden under compute)
            wkv = mp.tile([P, 3 * 2 * KH], bf16, name="wkv")
            bks = mp.tile([1, 3 * KH], bf16, name="bks")
            w1s = mp.tile([P, 72 * P], bf16, name="w1s")
            w2s = mp.tile([P, 36 * P], bf16, name="w2s")
            cst = mp.tile([P, 8], f32, name="cst")
            # BN scratch
            t1 = [mp.tile([P, 1], f32, name=f"t1_{i}") for i in range(2)]
            Ac = [mp.tile([P, 1], f32, name=f"Ac{i}") for i in range(2)]
            Bc = [mp.tile([P, 1], f32, name=f"Bc{i}") for i in range(2)]
            sq = [mp.tile([P, 1], f32, name=f"sq{i}") for i in range(2)]
            vart = [mp.tile([P, 1], f32, name=f"vart{i}") for i in range(2)]
            stdt = [mp.tile([P, 1], f32, name=f"stdt{i}") for i in range(2)]

            def copy_on(e, dst, src):
                if e is nc.scalar:
                    nc.scalar.copy(dst, src)
                else:
                    e.tensor_copy(dst, src)

            def load_kqv_weights(l):
                nc.sync.dma_start(wkv[:], wkqvd[l])
                nc.sync.dma_start(bks[:], bkd[l])

            # ---- init ----
            for i in range(2):
                nc.sync.dma_start(x[i][:], xin[i * P:(i + 1) * P, :])
                nc.scalar.copy(xb[i][:], x[i][:])
            nc.sync.dma_start(ident[:], identd)
            for i in range(4):
                nc.vector.memset(opad[i][:], 0)
            for i in range(2):
                nc.vector.memset(h1p[i][:], 0)
            nc.vector.memset(ones1[:], 1.0)
            load_kqv_weights(0)
            nc.sync.dma_start(w1s[:], w1d[0])
            nc.sync.dma_start(w2s[:], w2d[0])
            nc.sync.dma_start(cst[:], cstd[0])

            S3g = S[:].rearrange("p (t g) -> p g t", g=64)
            S3t = S[:].rearrange("p (t g) -> p t g", g=64)
            ab3g = attnb[:].rearrange("p (t g) -> p g t", g=64)
            ab3t = attnb[:].rearrange("p (t g) -> p t g", g=64)

            for l in range(layers):
                R, T = l + 1, l + 2
                bng = [cst[:, 4 * co + 0:4 * co + 1] for co in range(2)]
                bnb = [cst[:, 4 * co + 1:4 * co + 2] for co in range(2)]
                gob2 = [cst[:, 4 * co + 2:4 * co + 3] for co in range(2)]
                gam = [cst[:, 4 * co + 3:4 * co + 4] for co in range(2)]

                # ---- KQV (both halves; PE streams while copies drain) ----
                cnt = 0
                for hb in range(2):
                    for c, dest in ((0, kbt[l]), (1, vbt[l]), (2, qbt)):
                        for pbh in range(4):
                            pb = hb * 4 + pbh
                            ps = kqvps.tile([P, KH], f32, name="kqv_ps")
                            nc.tensor.matmul(ps[:], ones1[:],
                                             bks[0:1, c * KH:(c + 1) * KH],
                                             start=True, stop=False)
                            nc.tensor.matmul(ps[:], xb[0][:, pb * P:(pb + 1) * P],
                                             wkv[:, (2 * c + 0) * KH:(2 * c + 1) * KH],
                                             start=False, stop=False)
                            nc.tensor.matmul(ps[:], xb[1][:, pb * P:(pb + 1) * P],
                                             wkv[:, (2 * c + 1) * KH:(2 * c + 2) * KH],
                                             start=False, stop=True)
                            e = nc.vector if cnt % 3 == 2 else nc.scalar
                            copy_on(e, dest[:, pb * KH:(pb + 1) * KH], ps[:])
                            cnt += 1
                if l + 1 < layers:
                    load_kqv_weights(l + 1)

                def conv1_part(parts):
                    for co, ck in parts:
                        i0, nr = CHUNKS[ck]
                        nw = PW * nr
                        ps = convps.tile([P, 512], f32, name="c1ps", tag="cps")
                        for tap in range(9):
                            ty, tx = divmod(tap, 3)
                            base = PW * (i0 + ty) + tx
                            for ci in range(4):
                                off = ((co * 9 + tap) * 4 + ci) * P
                                nc.tensor.matmul(ps[:, 0:nw], w1s[:, off:off + P],
                                                 opad[ci][:, base:base + nw],
                                                 start=(tap == 0 and ci == 0),
                                                 stop=(tap == 8 and ci == 3))
                        e = nc.vector if ck % 2 else nc.scalar
                        copy_on(e, y1[co][:, W * i0:W * (i0 + nr)].rearrange(
                                    "c (i j) -> c i j", j=W),
                                ps[:, 0:nw].rearrange("c (i j) -> c i j", j=PW)[:, :, 0:W])

                # ---- attention halves, pipelined with transpose + conv1 ----
                for hb in range(2):
                    CL = slice(hb * 2048, (hb + 1) * 2048)
                    GS = slice(hb * 32, hb * 32 + 32)
                    # scores
                    for t in range(R):
                        pr = prodp.tile([P, 2048], bf16, name="prod", tag="pr")
                        e = nc.gpsimd if (R >= 3 and t == R - 1) else nc.vector
                        e.tensor_mul(pr[:], qbt[:, CL], kbt[t][:, CL])
                        nc.vector.tensor_reduce(
                            out=S3t[:, t, GS],
                            in_=pr[:].rearrange("p (g d) -> p g d", d=KD),
                            axis=AX.X, op=OP.add)
                    nc.vector.memset(S3t[:, R, GS], 0)  # zero-key slot
                    # softmax over T slots
                    nc.vector.tensor_reduce(out=mx[:, GS], in_=S3g[:, GS, 0:T],
                                            axis=AX.X, op=OP.max)
                    nc.vector.tensor_tensor(
                        ab3g[:, GS, 0:T], S3g[:, GS, 0:T],
                        mx[:, GS].unsqueeze(2).broadcast_to([P, 32, T]), OP.subtract)
                    nc.scalar.activation(ab3t[:, 0:T, GS], ab3t[:, 0:T, GS], ACTF.Exp)
                    nc.vector.tensor_reduce(out=zs[:, GS], in_=ab3g[:, GS, 0:T],
                                            axis=AX.X, op=OP.add)
                    nc.vector.reciprocal(zs[:, GS], zs[:, GS])
                    nc.vector.tensor_tensor(
                        ab3g[:, GS, 0:T], ab3g[:, GS, 0:T],
                        zs[:, GS].unsqueeze(2).broadcast_to([P, 32, T]), OP.mult)
                    # sparse top-k (T=5 only): delta = 2nd-smallest = 4th-largest
                    if T > TOPK:
                        first = True
                        for i in range(T):
                            for j in range(i + 1, T):
                                dst = dmin if first else mxp
                                nc.vector.tensor_tensor(dst[:, GS], ab3t[:, i, GS],
                                                        ab3t[:, j, GS], OP.max)
                                if not first:
                                    nc.vector.tensor_tensor(dmin[:, GS], dmin[:, GS],
                                                            mxp[:, GS], OP.min)
                                first = False
                        nc.vector.tensor_scalar_add(dmin[:, GS], dmin[:, GS], EPS)
                        nc.vector.tensor_tensor(
                            ab3g[:, GS, 0:T], ab3g[:, GS, 0:T],
                            dmin[:, GS].unsqueeze(2).broadcast_to([P, 32, T]),
                            OP.subtract)
                        nc.vector.tensor_scalar_max(ab3g[:, GS, 0:T],
                                                    ab3g[:, GS, 0:T], 0.0)
                        nc.vector.tensor_reduce(out=zs[:, GS], in_=ab3g[:, GS, 0:T],
                                                axis=AX.X, op=OP.add)
                        nc.vector.tensor_scalar_add(zs[:, GS], zs[:, GS], EPS)
                        nc.vector.reciprocal(zs[:, GS], zs[:, GS])
                        nc.vector.tensor_tensor(
                            ab3g[:, GS, 0:T], ab3g[:, GS, 0:T],
                            zs[:, GS].unsqueeze(2).broadcast_to([P, 32, T]), OP.mult)
                    # weighted sum -> obf half
                    for t in range(R):
                        abb = ab3t[:, t, GS].unsqueeze(2).broadcast_to([P, 32, KD])
                        vv = vbt[t][:, CL].rearrange("p (g d) -> p g d", d=KD)
                        e = nc.gpsimd if (R >= 3 and t == 0) else nc.vector
                        if t == 0:
                            e.tensor_tensor(
                                obf[:, CL].rearrange("p (g d) -> p g d", d=KD),
                                vv, abb, OP.mult)
                        else:
                            tm = prodp.tile([P, 2048], bf16, name="wtm", tag="pr")
                            e.tensor_tensor(tm[:].rearrange("p (g d) -> p g d", d=KD),
                                            vv, abb, OP.mult)
                            nc.vector.tensor_add(obf[:, CL], obf[:, CL], tm[:])
                    # PE transpose of this half into opad
                    for q in range(4):
                        tp = tpps.tile([P, 512], bf16, name="tp")
                        for pbh in range(4):
                            pb = hb * 4 + pbh
                            nc.tensor.matmul(
                                tp[:, pbh * P:(pbh + 1) * P],
                                obf[:, pb * KH + q * P: pb * KH + (q + 1) * P],
                                ident[:], is_transpose=True, skip_group_check=True)
                        opv = opad[q][:, 0:PHW].rearrange("c (i j) -> c i j", j=PW)
                        copy_on(nc.scalar,
                                opv[:, 1 + 16 * hb:17 + 16 * hb, 1:W + 1],
                                tp[:].rearrange("c (i j) -> c i j", j=W))
                    # conv1: chunk0 of co=0 overlaps attention of half 1
                    if hb == 0:
                        conv1_part([(0, 0)])

                conv1_part([(0, 1), (0, 2)])

                # ---- stats + AllGather per channel-half, pipelined ----
                def stats(i):
                    nc.vector.tensor_reduce(out=st[:, 2 * i:2 * i + 1], in_=y1[i][:],
                                            axis=AX.X, op=OP.add)
                    nc.scalar.square(sqs[i][:], y1[i][:])
                    nc.vector.tensor_reduce(out=st[:, 2 * i + 1:2 * i + 2],
                                            in_=sqs[i][:], axis=AX.X, op=OP.add)
                    cci = dramp.tile([1, 2 * P], f32, name="cci")
                    cco = dramp.tile([ncores, 2 * P], f32, name="cco",
                                     addr_space="Shared")
                    nc.sync.dma_start(cci[0].rearrange("(p j) -> p j", j=2),
                                      st[:, 2 * i:2 * i + 2])
                    nc.gpsimd.collective_compute(
                        "AllGather", OP.bypass,
                        replica_groups=[list(range(ncores))],
                        ins=[cci.opt()], outs=[cco.opt()])
                    nc.sync.dma_start(
                        gst[:, i * 16:(i + 1) * 16].rearrange("p (j s) -> p j s",
                                                              s=ncores),
                        cco[:].rearrange("s (p j) -> p j s", j=2))

                stats(0)
                conv1_part([(1, 0), (1, 1), (1, 2)])
                stats(1)
                if l + 1 < layers:
                    nc.sync.dma_start(w1s[:], w1d[l + 1])
                for co in range(2):
                    nc.scalar.add(x[co][:], x[co][:], gob2[co])

                # ---- BN coef + h1 + conv2 (ci-split overlaps AG latency) ----
                NTOT = float(ncores * HW)

                def bn_h1(i):
                    nc.vector.tensor_reduce(
                        out=gsum[:, 2 * i:2 * i + 2],
                        in_=gst[:, i * 16:(i + 1) * 16].rearrange(
                            "p (j s) -> p j s", s=ncores),
                        axis=AX.X, op=OP.add)
                    nc.vector.tensor_scalar_mul(t1[i][:], gsum[:, 2 * i:2 * i + 1],
                                                1.0 / NTOT)
                    nc.vector.tensor_scalar_mul(vart[i][:],
                                                gsum[:, 2 * i + 1:2 * i + 2],
                                                1.0 / NTOT)
                    nc.vector.tensor_mul(sq[i][:], t1[i][:], t1[i][:])
                    nc.vector.tensor_sub(vart[i][:], vart[i][:], sq[i][:])
                    nc.vector.tensor_scalar_add(vart[i][:], vart[i][:], BN_EPS)
                    nc.scalar.activation(stdt[i][:], vart[i][:], ACTF.Sqrt)
                    nc.vector.reciprocal(stdt[i][:], stdt[i][:])
                    nc.vector.tensor_mul(Ac[i][:], bng[i], stdt[i][:])
                    nc.vector.tensor_mul(sq[i][:], t1[i][:], Ac[i][:])
                    nc.vector.tensor_sub(Bc[i][:], bnb[i], sq[i][:])
                    h1v = h1p[i][:, 0:PHW].rearrange("c (i j) -> c i j", j=PW)
                    nc.scalar.activation(
                        h1v[:, 1:H + 1, 1:W + 1],
                        y1[i][:].rearrange("c (i j) -> c i j", j=W),
                        ACTF.Relu, bias=Bc[i][:], scale=Ac[i][:])

                def conv2_taps(ps2, co, ci, start, stop):
                    for ck, (i0, nr) in enumerate(CHUNKS):
                        nw = PW * nr
                        for tap in range(9):
                            ty, tx = divmod(tap, 3)
                            base = PW * (i0 + ty) + tx
                            off = ((co * 9 + tap) * 2 + ci) * P
                            nc.tensor.matmul(
                                ps2[ck][:, 0:nw], w2s[:, off:off + P],
                                h1p[ci][:, base:base + nw],
                                start=(start and tap == 0),
                                stop=(stop and tap == 8))

                def resid(ps2, co):
                    for ck, (i0, nr) in enumerate(CHUNKS):
                        nw = PW * nr
                        xsl = x[co][:, W * i0:W * (i0 + nr)].rearrange(
                            "c (i j) -> c i j", j=W)
                        nc.vector.scalar_tensor_tensor(
                            out=xsl,
                            in0=ps2[ck][:, 0:nw].rearrange(
                                "c (i j) -> c i j", j=PW)[:, :, 0:W],
                            scalar=gam[co], in1=xsl, op0=OP.mult, op1=OP.add)
                    if l < layers - 1:
                        nc.scalar.copy(xb[co][:], x[co][:])
                    else:
                        nc.sync.dma_start(out[co * P:(co + 1) * P, :], x[co][:])

                # co=0 split by ci so its ci=0 taps hide AG(1) latency
                bn_h1(0)
                ps20 = [convps.tile([P, 512], f32, name="c2ps", tag="cps")
                        for _ in range(3)]
                conv2_taps(ps20, 0, 0, True, False)
                bn_h1(1)
                conv2_taps(ps20, 0, 1, False, True)
                resid(ps20, 0)
                ps21 = [convps.tile([P, 512], f32, name="c2ps", tag="cps")
                        for _ in range(3)]
                conv2_taps(ps21, 1, 0, True, False)
                conv2_taps(ps21, 1, 1, False, True)
                if l + 1 < layers:
                    nc.sync.dma_start(w2s[:], w2d[l + 1])
                resid(ps21, 1)
                if l + 1 < layers:
                    nc.sync.dma_start(cst[:], cstd[l + 1])

    nc.compile()
    return nc


def _host_prep(inputs):
    bf = ml_dtypes.bfloat16
    kw, kb = inputs["kw"], inputs["kb"]
    qw, qb = inputs["qw"], inputs["qb"]
    vw, vb = inputs["vw"], inputs["vb"]
    ow1, ow2 = inputs["ow1"], inputs["ow2"]
    ob2, gammas = inputs["ob2"], inputs["gammas"]

    def packkqv(w):  # [L, KH, C] -> [L, P, 2, KH]
        return w.reshape(L, KH, 2, P).transpose(0, 3, 2, 1)

    d = {}
    wk3 = np.stack([packkqv(kw), packkqv(vw), packkqv(qw / 8.0)], axis=2)
    d["wkqvd"] = np.ascontiguousarray(wk3.reshape(L, P, 3 * 2 * KH)).astype(bf)
    bk3 = np.stack([kb, vb, qb / 8.0], axis=1)
    d["bkd"] = np.ascontiguousarray(bk3.reshape(L, 1, 3 * KH)).astype(bf)
    # ow1 [L, co*P+oc, ci*P+p, ty, tx] -> [L, p, co, ty, tx, ci, oc]
    a1 = ow1.reshape(L, 2, P, 4, P, 3, 3).transpose(0, 4, 1, 5, 6, 3, 2)
    d["w1d"] = np.ascontiguousarray(a1.reshape(L, P, 72 * P)).astype(bf)
    a2 = ow2.reshape(L, 2, P, 2, P, 3, 3).transpose(0, 4, 1, 5, 6, 3, 2)
    d["w2d"] = np.ascontiguousarray(a2.reshape(L, P, 36 * P)).astype(bf)
    cstv = np.zeros((L, 2, P, 4), np.float32)
    cstv[..., 0] = inputs["bn_g"].reshape(L, 2, P)
    cstv[..., 1] = inputs["bn_b"].reshape(L, 2, P)
    cstv[..., 2] = (gammas[:, None] * ob2).reshape(L, 2, P)
    cstv[..., 3] = gammas[:, None, None]
    d["cstd"] = np.ascontiguousarray(
        cstv.transpose(0, 2, 1, 3).reshape(L, P, 8)).astype(np.float32)
    d["identd"] = np.eye(P, dtype=np.float32).astype(bf)
    return d


def kernel(**inputs):
    if "nc" not in _compiled:
        _compiled["nc"] = _build()
    nc = _compiled["nc"]
    shared = _host_prep(inputs)
    x = np.ascontiguousarray(inputs["x"].reshape(B, C, HW)).astype(np.float32)
    in_maps = []
    for c in range(NC):
        m = dict(shared)
        m["xin"] = x[c]
        in_maps.append(m)
    res = bass_utils.run_bass_kernel_spmd(nc, in_maps, core_ids=list(range(NC)))
    outs = np.stack([res.results[c]["out"] for c in range(NC)])
    return outs.reshape(B, C, H, W).astype(np.float32)


# revision 13
# speedup vs baseline: 2.5644x; 1.1138x over previous
"""AttentiveDensenet Trainium2 Bass kernel (v2).

Data-parallel over batch B=8 across 8 NeuronCores (1 image per core).

Key structure (vs v1): all conv/KQV weights are DMA'd in ONE batched
transfer per (layer, tensor) from host-packed layouts (the v1 per-tile
weight DMAs cost ~850us of sync-sequencer DIRECT2D issue time and
starved the PE into mid-pstate). The o pos-major -> channel-major
transpose is done on the PE (is_transpose matmuls against an identity)
instead of a DRAM xbar bounce. Attention arithmetic runs in bf16
(DVE 2x mode; validated 1.34e-2 rel err vs 2e-2 gate), split between
DVE and GpSimd, and is processed in two position-halves so conv1
chunk 0 and the o-transpose of half 0 overlap with attention of
half 1. BN AllGathers (one per channel-half) overlap conv1 of the
other half and conv2's ci=0 taps via a channel-half-split pipeline.

Per layer l (of 4):
  - K/Q/V 1x1 convs as bf16 matmuls, x-tiles stationary, position-major
    [pos, (head, dim)] output; bias via ones-row matmul into PSUM.
  - scores/softmax/top-k/weighted-sum on DVE (+GpSimd offload), bf16.
  - o transposed on PE into zero-padded conv input tiles.
  - conv3x3 #1 as 9 shifted 1x1 matmuls accumulated in PSUM.
  - BN stats (sum/sumsq) AllGathered across 8 cores (2KB); ob1 cancels
    in training-mode BN and is dropped. h1 = relu(A*y1+B) fused on Act.
  - conv3x3 #2 + residual x += gamma*(h2 + ob2) via scalar_tensor_tensor.
"""
import numpy as np
import ml_dtypes

import concourse.bacc as bacc
import concourse.mybir as mybir
import concourse.tile as tile
from concourse import bass_utils

L, C, B, H, W = 4, 256, 8, 32, 32
NH, KD = 8, 64
KH = NH * KD          # 512
HW = H * W            # 1024
P = 128
NC = 8                # cores
TOPK = 4
EPS = 1e-7
BN_EPS = 1e-5
PW = W + 2            # 34
PHW = PW * (H + 2)    # 1156
CHUNKS = [(0, 15), (15, 15), (30, 2)]

f32 = mybir.dt.float32
bf16 = mybir.dt.bfloat16
AX = mybir.AxisListType
OP = mybir.AluOpType
ACTF = mybir.ActivationFunctionType

_compiled = {}


def _build(ncores=NC, layers=L):
    nc = bacc.Bacc(None, target_bir_lowering=False, debug=False, num_devices=ncores)

    xin = nc.dram_tensor("xin", [C, HW], f32, kind="ExternalInput").ap()
    wkqvd = nc.dram_tensor("wkqvd", [L, P, 3 * 2 * KH], bf16, kind="ExternalInput").ap()
    bkd = nc.dram_tensor("bkd", [L, 1, 3 * KH], bf16, kind="ExternalInput").ap()
    w1d = nc.dram_tensor("w1d", [L, P, 72 * P], bf16, kind="ExternalInput").ap()
    w2d = nc.dram_tensor("w2d", [L, P, 36 * P], bf16, kind="ExternalInput").ap()
    cstd = nc.dram_tensor("cstd", [L, P, 8], f32, kind="ExternalInput").ap()
    identd = nc.dram_tensor("identd", [P, P], bf16, kind="ExternalInput").ap()
    out = nc.dram_tensor("out", [C, HW], f32, kind="ExternalOutput").ap()

    with tile.TileContext(nc) as tc, \
         nc.allow_low_precision(reason="bf16 attention validated vs reference"):
        with tc.tile_pool(name="main", bufs=1) as mp, \
             tc.tile_pool(name="prodp", bufs=4) as prodp, \
             tc.tile_pool(name="psp", bufs=6, space="PSUM") as psp, \
             tc.tile_pool(name="tpps", bufs=2, space="PSUM") as tpps, \
             tc.tile_pool(name="dramp", bufs=4, space="DRAM") as dramp:

            # ---- persistent tiles ----
            x = [mp.tile([P, HW], f32, name=f"x{i}") for i in range(2)]
            xb = [mp.tile([P, HW], bf16, name=f"xb{i}") for i in range(2)]
            qbt = mp.tile([P, 8 * KH], bf16, name="qbt")
            kbt = [mp.tile([P, 8 * KH], bf16, name=f"kbt{i}") for i in range(L)]
            vbt = [mp.tile([P, 8 * KH], bf16, name=f"vbt{i}") for i in range(L)]
            obf = mp.tile([P, 8 * KH], bf16, name="obf")
            S = mp.tile([P, 5 * 64], f32, name="S")
            attnb = mp.tile([P, 5 * 64], bf16, name="attnb")
            attn = mp.tile([P, 5 * 64], f32, name="attn")
            mx = mp.tile([P, 64], f32, name="mx")
            zs = mp.tile([P, 64], f32, name="zs")
            dmin = mp.tile([P, 64], f32, name="dmin")
            mxp = mp.tile([P, 64], f32, name="mxp")
            opad = [mp.tile([P, PHW + 2], bf16, name=f"opad{i}") for i in range(4)]
            y1 = [mp.tile([P, HW], f32, name=f"y1_{i}") for i in range(2)]
            sqs = [mp.tile([P, HW], f32, name=f"sqs{i}") for i in range(2)]
            h1p = [mp.tile([P, PHW + 2], bf16, name=f"h1p{i}") for i in range(2)]
            st = mp.tile([P, 4], f32, name="st")
            gst = mp.tile([P, 32], f32, name="gst")
            gsum = mp.tile([P, 4], f32, name="gsum")
            ones1 = mp.tile([1, P], bf16, name="ones1")
            ident = mp.tile([P, P], bf16, name="ident")
            # weight buffers (single-buffered; each reloads for layer l+1
            # right after its last layer-l consumer, hidden under compute)
            wkv = mp.tile([P, 3 * 2 * KH], bf16, name="wkv")
            bks = mp.tile([1, 3 * KH], bf16, name="bks")
            w1s = mp.tile([P, 72 * P], bf16, name="w1s")
            w2s = mp.tile([P, 36 * P], bf16, name="w2s")
            cst = mp.tile([P, 8], f32, name="cst")
            # BN scratch
            t1 = [mp.tile([P, 1], f32, name=f"t1_{i}") for i in range(2)]
            Ac = [mp.tile([P, 1], f32, name=f"Ac{i}") for i in range(2)]
            Bc = [mp.tile([P, 1], f32, name=f"Bc{i}") for i in range(2)]
            sq = [mp.tile([P, 1], f32, name=f"sq{i}") for i in range(2)]
            vart = [mp.tile([P, 1], f32, name=f"vart{i}") for i in range(2)]
            stdt = [mp.tile([P, 1], f32, name=f"stdt{i}") for i in range(2)]

            def copy_on(e, dst, src):
                if e is nc.scalar:
                    nc.scalar.copy(dst, src)
                else:
                    e.tensor_copy(dst, src)

            def load_kqv_weights(l):
                nc.sync.dma_start(wkv[:], wkqvd[l])
                nc.sync.dma_start(bks[:], bkd[l])

            # ---- init ----
            for i in range(2):
                nc.sync.dma_start(x[i][:], xin[i * P:(i + 1) * P, :])
                nc.scalar.copy(xb[i][:], x[i][:])
            nc.sync.dma_start(ident[:], identd)
            for i in range(4):
                nc.vector.memset(opad[i][:], 0)
            for i in range(2):
                nc.vector.memset(h1p[i][:], 0)
            nc.vector.memset(ones1[:], 1.0)
            load_kqv_weights(0)
            nc.sync.dma_start(w1s[:], w1d[0])
            nc.sync.dma_start(w2s[:], w2d[0])
            nc.sync.dma_start(cst[:], cstd[0])

            S3g = S[:].rearrange("p (t g) -> p g t", g=64)
            S3t = S[:].rearrange("p (t g) -> p t g", g=64)
            ab3g = attn[:].rearrange("p (t g) -> p g t", g=64)
            ab3t = attn[:].rearrange("p (t g) -> p t g", g=64)
            abb3t = attnb[:].rearrange("p (t g) -> p t g", g=64)

            for l in range(layers):
                R, T = l + 1, l + 2
                bng = [cst[:, 4 * co + 0:4 * co + 1] for co in range(2)]
                bnb = [cst[:, 4 * co + 1:4 * co + 2] for co in range(2)]
                gob2 = [cst[:, 4 * co + 2:4 * co + 3] for co in range(2)]
                gam = [cst[:, 4 * co + 3:4 * co + 4] for co in range(2)]

                # ---- KQV (both halves; PE streams while copies drain) ----
                cnt = 0
                for hb in range(2):
                    for c, dest in ((0, kbt[l]), (1, vbt[l]), (2, qbt)):
                        for pbh in range(4):
                            pb = hb * 4 + pbh
                            ps = psp.tile([P, KH], f32, name="ps", tag="ps")
                            nc.tensor.matmul(ps[:], ones1[:],
                                             bks[0:1, c * KH:(c + 1) * KH],
                                             start=True, stop=False)
                            nc.tensor.matmul(ps[:], xb[0][:, pb * P:(pb + 1) * P],
                                             wkv[:, (2 * c + 0) * KH:(2 * c + 1) * KH],
                                             start=False, stop=False)
                            nc.tensor.matmul(ps[:], xb[1][:, pb * P:(pb + 1) * P],
                                             wkv[:, (2 * c + 1) * KH:(2 * c + 2) * KH],
                                             start=False, stop=True)
                            e = nc.vector if cnt % 4 == 3 else nc.scalar
                            copy_on(e, dest[:, pb * KH:(pb + 1) * KH], ps[:])
                            cnt += 1
                if l + 1 < layers:
                    load_kqv_weights(l + 1)

                def conv1_part(parts):
                    for co, ck in parts:
                        i0, nr = CHUNKS[ck]
                        nw = PW * nr
                        ps = psp.tile([P, 512], f32, name="ps", tag="ps")
                        for tap in range(9):
                            ty, tx = divmod(tap, 3)
                            base = PW * (i0 + ty) + tx
                            for ci in range(4):
                                off = ((co * 9 + tap) * 4 + ci) * P
                                nc.tensor.matmul(ps[:, 0:nw], w1s[:, off:off + P],
                                                 opad[ci][:, base:base + nw],
                                                 start=(tap == 0 and ci == 0),
                                                 stop=(tap == 8 and ci == 3))
                        e = nc.vector if ck % 2 else nc.scalar
                        copy_on(e, y1[co][:, W * i0:W * (i0 + nr)].rearrange(
                                    "c (i j) -> c i j", j=W),
                                ps[:, 0:nw].rearrange("c (i j) -> c i j", j=PW)[:, :, 0:W])

                # ---- attention halves, pipelined with transpose + conv1 ----
                for hb in range(2):
                    CL = slice(hb * 2048, (hb + 1) * 2048)
                    GS = slice(hb * 32, hb * 32 + 32)
                    # scores
                    for t in range(R):
                        pr = prodp.tile([P, 2048], bf16, name="prod", tag="pr")
                        nc.vector.tensor_mul(pr[:], qbt[:, CL], kbt[t][:, CL])
                        nc.vector.tensor_reduce(
                            out=S3t[:, t, GS],
                            in_=pr[:].rearrange("p (g d) -> p g d", d=KD),
                            axis=AX.X, op=OP.add)
                    nc.vector.memset(S3t[:, R, GS], 0)  # zero-key slot
                    # softmax over T slots
                    nc.vector.tensor_reduce(out=mx[:, GS], in_=S3g[:, GS, 0:T],
                                            axis=AX.X, op=OP.max)
                    nc.vector.tensor_tensor(
                        ab3g[:, GS, 0:T], S3g[:, GS, 0:T],
                        mx[:, GS].unsqueeze(2).broadcast_to([P, 32, T]), OP.subtract)
                    nc.scalar.activation(ab3t[:, 0:T, GS], ab3t[:, 0:T, GS], ACTF.Exp)
                    nc.vector.tensor_reduce(out=zs[:, GS], in_=ab3g[:, GS, 0:T],
                                            axis=AX.X, op=OP.add)
                    nc.vector.reciprocal(zs[:, GS], zs[:, GS])
                    nc.vector.tensor_tensor(
                        ab3g[:, GS, 0:T], ab3g[:, GS, 0:T],
                        zs[:, GS].unsqueeze(2).broadcast_to([P, 32, T]), OP.mult)
                    # sparse top-k (T=5 only): delta = 2nd-smallest = 4th-largest
                    if T > TOPK:
                        first = True
                        for i in range(T):
                            for j in range(i + 1, T):
                                dst = dmin if first else mxp
                                nc.vector.tensor_tensor(dst[:, GS], ab3t[:, i, GS],
                                                        ab3t[:, j, GS], OP.max)
                                if not first:
                                    nc.vector.tensor_tensor(dmin[:, GS], dmin[:, GS],
                                                            mxp[:, GS], OP.min)
                                first = False
                        nc.vector.tensor_scalar_add(dmin[:, GS], dmin[:, GS], EPS)
                        nc.vector.tensor_tensor(
                            ab3g[:, GS, 0:T], ab3g[:, GS, 0:T],
                            dmin[:, GS].unsqueeze(2).broadcast_to([P, 32, T]),
                            OP.subtract)
                        nc.vector.tensor_scalar_max(ab3g[:, GS, 0:T],
                                                    ab3g[:, GS, 0:T], 0.0)
                        nc.vector.tensor_reduce(out=zs[:, GS], in_=ab3g[:, GS, 0:T],
                                                axis=AX.X, op=OP.add)
                        nc.vector.tensor_scalar_add(zs[:, GS], zs[:, GS], EPS)
                        nc.vector.reciprocal(zs[:, GS], zs[:, GS])
                        nc.vector.tensor_tensor(
                            ab3g[:, GS, 0:T], ab3g[:, GS, 0:T],
                            zs[:, GS].unsqueeze(2).broadcast_to([P, 32, T]), OP.mult)
                    nc.vector.tensor_copy(abb3t[:, 0:T, GS], ab3t[:, 0:T, GS])
                    # weighted sum -> obf half
                    for t in range(R):
                        abb = abb3t[:, t, GS].unsqueeze(2).broadcast_to([P, 32, KD])
                        vv = vbt[t][:, CL].rearrange("p (g d) -> p g d", d=KD)
                        if t == 0:
                            nc.vector.tensor_tensor(
                                obf[:, CL].rearrange("p (g d) -> p g d", d=KD),
                                vv, abb, OP.mult)
                        else:
                            tm = prodp.tile([P, 2048], bf16, name="wtm", tag="pr")
                            nc.vector.tensor_tensor(
                                tm[:].rearrange("p (g d) -> p g d", d=KD),
                                vv, abb, OP.mult)
                            nc.vector.tensor_add(obf[:, CL], obf[:, CL], tm[:])
                    # PE transpose of this half into opad
                    for q in range(4):
                        tp = tpps.tile([P, 512], bf16, name="tp")
                        for pbh in range(4):
                            pb = hb * 4 + pbh
                            nc.tensor.matmul(
                                tp[:, pbh * P:(pbh + 1) * P],
                                obf[:, pb * KH + q * P: pb * KH + (q + 1) * P],
                                ident[:], is_transpose=True, skip_group_check=True)
                        opv = opad[q][:, 0:PHW].rearrange("c (i j) -> c i j", j=PW)
                        copy_on(nc.scalar,
                                opv[:, 1 + 16 * hb:17 + 16 * hb, 1:W + 1],
                                tp[:].rearrange("c (i j) -> c i j", j=W))
                    # conv1: chunk0 of co=0 overlaps attention of half 1
                    if hb == 0:
                        conv1_part([(0, 0)])

                conv1_part([(0, 1), (0, 2)])

                # ---- stats + AllGather per channel-half, pipelined ----
                def stats(i):
                    nc.vector.tensor_reduce(out=st[:, 2 * i:2 * i + 1], in_=y1[i][:],
                                            axis=AX.X, op=OP.add)
                    nc.scalar.square(sqs[i][:], y1[i][:])
                    nc.vector.tensor_reduce(out=st[:, 2 * i + 1:2 * i + 2],
                                            in_=sqs[i][:], axis=AX.X, op=OP.add)
                    cci = dramp.tile([1, 2 * P], f32, name="cci")
                    cco = dramp.tile([ncores, 2 * P], f32, name="cco",
                                     addr_space="Shared")
                    nc.sync.dma_start(cci[0].rearrange("(p j) -> p j", j=2),
                                      st[:, 2 * i:2 * i + 2])
                    nc.gpsimd.collective_compute(
                        "AllGather", OP.bypass,
                        replica_groups=[list(range(ncores))],
                        ins=[cci.opt()], outs=[cco.opt()])
                    nc.sync.dma_start(
                        gst[:, i * 16:(i + 1) * 16].rearrange("p (j s) -> p j s",
                                                              s=ncores),
                        cco[:].rearrange("s (p j) -> p j s", j=2))

                stats(0)
                conv1_part([(1, 0), (1, 1), (1, 2)])
                stats(1)
                if l + 1 < layers:
                    nc.sync.dma_start(w1s[:], w1d[l + 1])
                for co in range(2):
                    nc.scalar.add(x[co][:], x[co][:], gob2[co])

                # ---- BN coef + h1 + conv2 (ci-split overlaps AG latency) ----
                NTOT = float(ncores * HW)

                def bn_h1(i):
                    nc.vector.tensor_reduce(
                        out=gsum[:, 2 * i:2 * i + 2],
                        in_=gst[:, i * 16:(i + 1) * 16].rearrange(
                            "p (j s) -> p j s", s=ncores),
                        axis=AX.X, op=OP.add)
                    nc.vector.tensor_scalar_mul(t1[i][:], gsum[:, 2 * i:2 * i + 1],
                                                1.0 / NTOT)
                    nc.vector.tensor_scalar_mul(vart[i][:],
                                                gsum[:, 2 * i + 1:2 * i + 2],
                                                1.0 / NTOT)
                    nc.vector.tensor_mul(sq[i][:], t1[i][:], t1[i][:])
                    nc.vector.tensor_sub(vart[i][:], vart[i][:], sq[i][:])
                    nc.vector.tensor_scalar_add(vart[i][:], vart[i][:], BN_EPS)
                    nc.scalar.activation(stdt[i][:], vart[i][:], ACTF.Sqrt)
                    nc.vector.reciprocal(stdt[i][:], stdt[i][:])
                    nc.vector.tensor_mul(Ac[i][:], bng[i], stdt[i][:])
                    nc.vector.tensor_mul(sq[i][:], t1[i][:], Ac[i][:])
                    nc.vector.tensor_sub(Bc[i][:], bnb[i], sq[i][:])
                    h1v = h1p[i][:, 0:PHW].rearrange("c (i j) -> c i j", j=PW)
                    nc.scalar.activation(
                        h1v[:, 1:H + 1, 1:W + 1],
                        y1[i][:].rearrange("c (i j) -> c i j", j=W),
                        ACTF.Relu, bias=Bc[i][:], scale=Ac[i][:])

                def conv2_taps(ps2, co, ci, start, stop):
                    for ck, (i0, nr) in enumerate(CHUNKS):
                        nw = PW * nr
                        for tap in range(9):
                            ty, tx = divmod(tap, 3)
                            base = PW * (i0 + ty) + tx
                            off = ((co * 9 + tap) * 2 + ci) * P
                            nc.tensor.matmul(
                                ps2[ck][:, 0:nw], w2s[:, off:off + P],
                                h1p[ci][:, base:base + nw],
                                start=(start and tap == 0),
                                stop=(stop and tap == 8))

                def resid(ps2, co):
                    for ck, (i0, nr) in enumerate(CHUNKS):
                        nw = PW * nr
                        xsl = x[co][:, W * i0:W * (i0 + nr)].rearrange(
                            "c (i j) -> c i j", j=W)
                        nc.vector.scalar_tensor_tensor(
                            out=xsl,
                            in0=ps2[ck][:, 0:nw].rearrange(
                                "c (i j) -> c i j", j=PW)[:, :, 0:W],
                            scalar=gam[co], in1=xsl, op0=OP.mult, op1=OP.add)
                    if l < layers - 1:
                        nc.scalar.copy(xb[co][:], x[co][:])
                    else:
                        nc.sync.dma_start(out[co * P:(co + 1) * P, :], x[co][:])

                # ci-split: both co groups' ci=0 taps hide AG(1) latency
                bn_h1(0)
                ps20 = [psp.tile([P, 512], f32, name="ps", tag="ps")
                        for _ in range(3)]
                ps21 = [psp.tile([P, 512], f32, name="ps", tag="ps")
                        for _ in range(3)]
                conv2_taps(ps20, 0, 0, True, False)
                conv2_taps(ps21, 1, 0, True, False)
                bn_h1(1)
                conv2_taps(ps20, 0, 1, False, True)
                resid(ps20, 0)
                conv2_taps(ps21, 1, 1, False, True)
                if l + 1 < layers:
                    nc.sync.dma_start(w2s[:], w2d[l + 1])
                resid(ps21, 1)
                if l + 1 < layers:
                    nc.sync.dma_start(cst[:], cstd[l + 1])

    nc.compile()
    return nc


def _host_prep(inputs):
    bf = ml_dtypes.bfloat16
    kw, kb = inputs["kw"], inputs["kb"]
    qw, qb = inputs["qw"], inputs["qb"]
    vw, vb = inputs["vw"], inputs["vb"]
    ow1, ow2 = inputs["ow1"], inputs["ow2"]
    ob2, gammas = inputs["ob2"], inputs["gammas"]

    def packkqv(w):  # [L, KH, C] -> [L, P, 2, KH]
        return w.reshape(L, KH, 2, P).transpose(0, 3, 2, 1)

    d = {}
    wk3 = np.stack([packkqv(kw), packkqv(vw), packkqv(qw / 8.0)], axis=2)
    d["wkqvd"] = np.ascontiguousarray(wk3.reshape(L, P, 3 * 2 * KH)).astype(bf)
    bk3 = np.stack([kb, vb, qb / 8.0], axis=1)
    d["bkd"] = np.ascontiguousarray(bk3.reshape(L, 1, 3 * KH)).astype(bf)
    # ow1 [L, co*P+oc, ci*P+p, ty, tx] -> [L, p, co, ty, tx, ci, oc]
    a1 = ow1.reshape(L, 2, P, 4, P, 3, 3).transpose(0, 4, 1, 5, 6, 3, 2)
    d["w1d"] = np.ascontiguousarray(a1.reshape(L, P, 72 * P)).astype(bf)
    a2 = ow2.reshape(L, 2, P, 2, P, 3, 3).transpose(0, 4, 1, 5, 6, 3, 2)
    d["w2d"] = np.ascontiguousarray(a2.reshape(L, P, 36 * P)).astype(bf)
    cstv = np.zeros((L, 2, P, 4), np.float32)
    cstv[..., 0] = inputs["bn_g"].reshape(L, 2, P)
    cstv[..., 1] = inputs["bn_b"].reshape(L, 2, P)
    cstv[..., 2] = (gammas[:, None] * ob2).reshape(L, 2, P)
    cstv[..., 3] = gammas[:, None, None]
    d["cstd"] = np.ascontiguousarray(
        cstv.transpose(0, 2, 1, 3).reshape(L, P, 8)).astype(np.float32)
    d["identd"] = np.eye(P, dtype=np.float32).astype(bf)
    return d


def kernel(**inputs):
    if "nc" not in _compiled:
        _compiled["nc"] = _build()
    nc = _compiled["nc"]
    shared = _host_prep(inputs)
    x = np.ascontiguousarray(inputs["x"].reshape(B, C, HW)).astype(np.float32)
    in_maps = []
    for c in range(NC):
        m = dict(shared)
        m["xin"] = x[c]
        in_maps.append(m)
    res = bass_utils.run_bass_kernel_spmd(nc, in_maps, core_ids=list(range(NC)))
    outs = np.stack([res.results[c]["out"] for c in range(NC)])
    return outs.reshape(B, C, H, W).astype(np.float32)
